# revision 1
# baseline (speedup 1.0000x reference)
"""Cross-attention block kernel for Trainium2 (8 NeuronCores, data-parallel).

Computes, for full inputs:
    Q = x @ Wq + bq            [B, HW, D]
    K = a @ Wk + bk            [B, S, D]
    V = a @ Wv + bv            [B, S, D]
    out = softmax(Q K^T / sqrt(D)) @ V

Sharding: batch (B=16) split across 8 cores, 2 batches per core. Weights
replicated. No collectives needed.

Per-core kernel strategy (all matmuls in float32r = full PE rate, FP22):
  - PE-transpose x and audio tiles into d-major SBUF layouts (xT, aT).
  - qT = Wq^T @ xT   (out [d_out-part, hw-free]; bias added by ACT copy)
  - kT = Wk^T @ aT   (out [d_out-part, s-free])
  - v  = aT^T @ Wv   (out [s-part, d-free]; bias added by DVE)
  - scoresT[s, hw] = (kT chunk)^T @ qT, accumulated over d in PSUM;
    ACT computes exp(scale * scoresT) straight out of PSUM (no max
    subtraction: scores have std ~0.33, max |score| < ~3, exp is safe).
  - out[hw, d] = sum_s expT^T @ V with an interleaved ones-column matmul
    accumulating the softmax denominator in a second PSUM bank; the
    final ACT copy applies the reciprocal as a per-partition scale.

float32r is a 4-byte fp32 view that the PE truncates to FP22; the walrus
verifier requires every producer feeding an FP32r matmul to emit float32r,
so the whole operand chain (DRAM tensors included) is declared float32r.
"""

from contextlib import ExitStack

import numpy as np

import concourse.bass as bass
import concourse.bacc as bacc
import concourse.mybir as mybir
import concourse.tile as tile
from concourse.bass_utils import run_bass_kernel_spmd
from concourse.masks import make_identity

P = 128
D = 512          # d_query == d_audio == d_out
CD = D // P      # 4 chunks of the feature dim
HW = 4096        # queries per batch
S = 1024         # keys per batch
SC = S // P      # 8 s-chunks
HWB = 512        # hw rows processed per block
NBLK = HW // HWB
B_FULL = 16
N_CORES = 8
BL = B_FULL // N_CORES  # 2 batches per core
SCALE = 1.0 / float(np.sqrt(D))

f32 = mybir.dt.float32
f32r = mybir.dt.float32r
AFT = mybir.ActivationFunctionType


def build_nc():
    nc = bacc.Bacc("TRN2", target_bir_lowering=False, debug=False)

    x = nc.dram_tensor("x", [BL, HW, D], f32r, kind="ExternalInput").ap()
    audio = nc.dram_tensor("audio_embed", [BL, S, D], f32r, kind="ExternalInput").ap()
    wq = nc.dram_tensor("Wq", [D, D], f32r, kind="ExternalInput").ap()
    bq = nc.dram_tensor("bq", [D], f32, kind="ExternalInput").ap()
    wk = nc.dram_tensor("Wk", [D, D], f32r, kind="ExternalInput").ap()
    bk = nc.dram_tensor("bk", [D], f32, kind="ExternalInput").ap()
    wv = nc.dram_tensor("Wv", [D, D], f32r, kind="ExternalInput").ap()
    bv = nc.dram_tensor("bv", [D], f32r, kind="ExternalInput").ap()
    out = nc.dram_tensor("out", [BL, HW, D], f32, kind="ExternalOutput").ap()

    with tile.TileContext(nc) as tc:
        with ExitStack() as ctx:
            _body(ctx, tc, x, audio, wq, bq, wk, bk, wv, bv, out)

    nc.compile()
    return nc


def _body(ctx, tc, x, audio, wq, bq, wk, bk, wv, bv, out):
    nc = tc.nc

    const_pool = ctx.enter_context(tc.tile_pool(name="const", bufs=1))
    batch_pool = ctx.enter_context(tc.tile_pool(name="batch", bufs=1))
    work_pool = ctx.enter_context(tc.tile_pool(name="work", bufs=2))
    small_pool = ctx.enter_context(tc.tile_pool(name="small", bufs=4))
    psum_tp = ctx.enter_context(tc.tile_pool(name="ptp", bufs=2, space="PSUM"))
    psum_mm = ctx.enter_context(tc.tile_pool(name="pmm", bufs=2, space="PSUM"))
    psum_sc = ctx.enter_context(tc.tile_pool(name="psc", bufs=2, space="PSUM"))
    psum_den = ctx.enter_context(tc.tile_pool(name="pden", bufs=2, space="PSUM"))

    # --- constants -----------------------------------------------------
    # gpsimd/iota writes are f32; launder through a DVE copy into f32r so
    # the BIR verifier sees a rounding producer for every matmul operand.
    ident_f = const_pool.tile([P, P], f32)
    make_identity(nc, ident_f)
    ident = const_pool.tile([P, P], f32r)
    nc.vector.tensor_copy(ident, ident_f)

    ones_f = const_pool.tile([P, 2], f32)
    nc.gpsimd.memset(ones_f, 1.0)
    ones_col = const_pool.tile([P, 2], f32r)
    nc.vector.tensor_copy(ones_col, ones_f)
    ones_row_f = const_pool.tile([1, P], f32)
    nc.gpsimd.memset(ones_row_f, 1.0)
    ones_row = const_pool.tile([1, P], f32r)
    nc.vector.tensor_copy(ones_row, ones_row_f)

    # Weight/bias loads are emitted lazily (after the first audio-half DMA)
    # so the first transposable input data leads the serial DMA queue; this
    # removes a ~12us PE startup stall waiting behind 6.5MB of constants.
    consts = {}

    def _load_consts():
        # small tensors first (bias ACT + bv broadcast gate PSUM drains),
        # then weights in first-use order; wq is emitted separately after
        # the x block-0 prefetch.
        bv_row = const_pool.tile([1, D], f32r)
        nc.sync.dma_start(bv_row, bv[None, :])
        bq_sb = const_pool.tile([P, CD], f32)
        nc.sync.dma_start(bq_sb, bq.rearrange("(c p) -> p c", p=P))
        bk_sb = const_pool.tile([P, CD], f32)
        nc.sync.dma_start(bk_sb, bk.rearrange("(c p) -> p c", p=P))
        wk_sb = const_pool.tile([P, CD, D], f32r)
        nc.sync.dma_start(wk_sb, wk.rearrange("(c p) n -> p c n", p=P))
        wv_sb = const_pool.tile([P, CD, D], f32r)
        nc.sync.dma_start(wv_sb, wv.rearrange("(c p) n -> p c n", p=P))
        # bv broadcast to all 128 partitions via a K=1 outer-product matmul
        bv_ps = psum_mm.tile([P, D], f32, tag="mm")
        nc.tensor.matmul(bv_ps, ones_row, bv_row, start=True, stop=True)
        bv_bc = const_pool.tile([P, D], f32)
        nc.vector.tensor_copy(bv_bc, bv_ps)
        consts.update(wk_sb=wk_sb, wv_sb=wv_sb,
                      bq_sb=bq_sb, bk_sb=bk_sb, bv_bc=bv_bc)

    def _load_wq():
        wq_sb = const_pool.tile([P, CD, D], f32r)
        nc.sync.dma_start(wq_sb, wq.rearrange("(c p) n -> p c n", p=P))
        consts.update(wq_sb=wq_sb)

    x_pre = {}
    for b in range(BL):
        # --- per-batch: audio transpose, K^T, V; one audio half at a time
        # so half-0 compute never waits behind the half-1 DMA -----------
        aT = batch_pool.tile([P, CD, S], f32r, tag="aT")
        kT = batch_pool.tile([P, CD, S], f32r, tag="kT")
        v_sb = batch_pool.tile([P, SC, D], f32r, tag="v")
        for half in range(2):
            a_half = work_pool.tile([P, CD, D], f32r, tag="x")
            nc.sync.dma_start(
                a_half, audio[b].rearrange("(t c p) n -> t p c n", p=P, c=CD)[half]
            )
            if b == 0 and half == 0:
                _load_consts()
                x_pre0 = work_pool.tile([P, CD, D], f32r, tag="x", name="x_pre0")
                nc.sync.dma_start(
                    x_pre0, x[0].rearrange("(t c p) n -> t p c n", p=P, c=CD)[0]
                )
                x_pre[(0, 0)] = x_pre0
                _load_wq()
            for dc in range(CD):
                tp_ps = psum_tp.tile([P, HWB], f32r, tag="tp")
                for c in range(CD):
                    nc.tensor.matmul(
                        tp_ps[:, c * P : (c + 1) * P],
                        a_half[:, c, dc * P : (dc + 1) * P],
                        ident,
                        is_transpose=True,
                    )
                nc.vector.tensor_copy(aT[:, dc, half * 512 : (half + 1) * 512], tp_ps)

            for m in range(CD):
                mm_ps = psum_mm.tile([P, 512], f32, tag="mm")
                for c in range(CD):
                    nc.tensor.matmul(
                        mm_ps,
                        consts["wk_sb"][:, c, m * P : (m + 1) * P],
                        aT[:, c, half * 512 : (half + 1) * 512],
                        start=(c == 0),
                        stop=(c == CD - 1),
                    )
                nc.scalar.activation(
                    kT[:, m, half * 512 : (half + 1) * 512],
                    mm_ps,
                    AFT.Identity,
                    bias=consts["bk_sb"][:, m, None],
                    scale=1.0,
                )

            for g in range(half * 4, half * 4 + 4):
                mm_ps = psum_mm.tile([P, D], f32, tag="mm")
                for c in range(CD):
                    nc.tensor.matmul(
                        mm_ps,
                        aT[:, c, g * P : (g + 1) * P],
                        consts["wv_sb"][:, c, :],
                        start=(c == 0),
                        stop=(c == CD - 1),
                    )
                nc.vector.tensor_add(v_sb[:, g, :], mm_ps, consts["bv_bc"])

        # --- hw blocks -------------------------------------------------
        for blk in range(NBLK):
            x_sb = x_pre.pop((b, blk), None)
            if x_sb is None:
                x_sb = work_pool.tile([P, CD, D], f32r, tag="x")
                nc.sync.dma_start(
                    x_sb, x[b].rearrange("(t c p) n -> t p c n", p=P, c=CD)[blk]
                )

            xT = work_pool.tile([P, CD, HWB], f32r, tag="xT")
            for dc in range(CD):
                tp_ps = psum_tp.tile([P, HWB], f32r, tag="tp")
                for c in range(CD):
                    nc.tensor.matmul(
                        tp_ps[:, c * P : (c + 1) * P],
                        x_sb[:, c, dc * P : (dc + 1) * P],
                        ident,
                        is_transpose=True,
                    )
                nc.vector.tensor_copy(xT[:, dc, :], tp_ps)

            qT = work_pool.tile([P, CD, HWB], f32r, tag="qT")
            for m in range(CD):
                mm_ps = psum_mm.tile([P, HWB], f32, tag="mm")
                for c in range(CD):
                    nc.tensor.matmul(
                        mm_ps,
                        consts["wq_sb"][:, c, m * P : (m + 1) * P],
                        xT[:, c, :],
                        start=(c == 0),
                        stop=(c == CD - 1),
                    )
                nc.scalar.activation(
                    qT[:, m, :], mm_ps, AFT.Identity, bias=consts["bq_sb"][:, m, None], scale=1.0
                )

            ex = work_pool.tile([P, SC, HWB], f32r, tag="ex")
            for g in range(SC):
                sc_ps = psum_sc.tile([P, HWB], f32, tag="sc")
                for m in range(CD):
                    nc.tensor.matmul(
                        sc_ps,
                        kT[:, m, g * P : (g + 1) * P],
                        qT[:, m, :],
                        start=(m == 0),
                        stop=(m == CD - 1),
                    )
                nc.scalar.activation(
                    ex[:, g, :], sc_ps, AFT.Exp, bias=0.0, scale=SCALE
                )

            out_sb = work_pool.tile([P, CD, D], f32, tag="o")
            for h in range(CD):
                o_ps = psum_mm.tile([P, D], f32, tag="mm")
                d_ps = psum_den.tile([P, 2], f32, tag="den")
                for g in range(SC):
                    lhs = ex[:, g, h * P : (h + 1) * P]
                    nc.tensor.matmul(
                        o_ps, lhs, v_sb[:, g, :], start=(g == 0), stop=(g == SC - 1)
                    )
                    nc.tensor.matmul(
                        d_ps, lhs, ones_col, start=(g == 0), stop=(g == SC - 1)
                    )
                rec = small_pool.tile([P, 1], f32, tag="rec")
                nc.vector.reciprocal(rec, d_ps[:, 0:1])
                nc.scalar.activation(
                    out_sb[:, h, :], o_ps, AFT.Copy, bias=0.0, scale=rec
                )
            nc.sync.dma_start(
                out[b].rearrange("(t h p) n -> t p h n", p=P, h=CD)[blk], out_sb
            )


_NC_CACHE = None


def _get_nc():
    global _NC_CACHE
    if _NC_CACHE is None:
        _NC_CACHE = build_nc()
    return _NC_CACHE


def kernel(**inputs):
    x = np.ascontiguousarray(np.asarray(inputs["x"], dtype=np.float32))
    audio = np.ascontiguousarray(np.asarray(inputs["audio_embed"], dtype=np.float32))
    wq = np.ascontiguousarray(np.asarray(inputs["Wq"], dtype=np.float32))
    bq = np.ascontiguousarray(np.asarray(inputs["bq"], dtype=np.float32))
    wk = np.ascontiguousarray(np.asarray(inputs["Wk"], dtype=np.float32))
    bk = np.ascontiguousarray(np.asarray(inputs["bk"], dtype=np.float32))
    wv = np.ascontiguousarray(np.asarray(inputs["Wv"], dtype=np.float32))
    bv = np.ascontiguousarray(np.asarray(inputs["bv"], dtype=np.float32))

    nc = _get_nc()
    in_maps = []
    for i in range(N_CORES):
        in_maps.append(
            {
                "x": np.ascontiguousarray(x[i * BL : (i + 1) * BL]),
                "audio_embed": np.ascontiguousarray(audio[i * BL : (i + 1) * BL]),
                "Wq": wq,
                "bq": bq,
                "Wk": wk,
                "bk": bk,
                "Wv": wv,
                "bv": bv,
            }
        )
    res = run_bass_kernel_spmd(nc, in_maps, core_ids=list(range(N_CORES)))
    return np.concatenate([res.results[i]["out"] for i in range(N_CORES)], axis=0)



# revision 2
# speedup vs baseline: 1.4412x; 1.4412x over previous
"""Cross-attention block kernel for Trainium2 (8 NeuronCores, data-parallel).

Computes, for full inputs:
    Q = x @ Wq + bq            [B, HW, D]
    K = a @ Wk + bk            [B, S, D]
    V = a @ Wv + bv            [B, S, D]
    out = softmax(Q K^T / sqrt(D)) @ V

Sharding: batch (B=16) split across 8 cores, 2 batches per core. Weights
replicated. No collectives needed.

Per-core strategy (fp8 e4m3 DoubleRow for the attention contractions):
  - Host pre-work: x/audio/weights cast to bf16; Wq,Wk,Wv,bk,bv scaled by 4
    so the fp8 operands use the e4m3 normal range.
  - x and audio transposed to d-major via the DMA XBAR (dma_start_transpose,
    16x128 tiles) -- the PE does no transposes at all.
  - Projections run in bf16 (1 cycle/row, full PE rate): qT = Wq^T @ xT,
    kT = Wk^T @ aT, v = aT^T @ Wv (+bv via a K=1 ones-row matmul folded into
    the same PSUM group).
  - Q-bias folding: softmax(Q K^T) is invariant to per-query shifts, so
    Q stays raw and the per-key correction delta[s] = bq . K'[s] (K' = K+bk)
    is added via the ACT bias port at the exp stage. delta comes from tiny
    DoubleRow matmuls (ap_size=1) against an e4m3 copy of bq.
  - scores^T[s, hw] = kh^T qh in PLAIN e4m3 DoubleRow (2 k-tiles per
    instruction, 0.5 cycles/row -> 4x fp32r rate). Q/K quantization error is
    damped by the softmax scale (std of scaled scores ~0.33), total ~1%.
  - ex = exp(scale*scores + scale*delta + ln8) computed by ACT straight from
    PSUM, split into an e4m3 hi/lo pair (exh = DVE quantize, exl = DVE sub).
    The ln8 bias scales ex by 8 so the whole range [1.3, 48] is normal e4m3.
  - out = (exh+exl) @ (vh+vl) with the lo*lo term dropped: hi*hi uses
    k-tile-paired DoubleRow; the cross terms exl*vh and exh*vl share single
    DoubleRow instructions (slot0=lo*hi, slot1=hi*lo). 1.33x fp32r rate.
  - denominator: DoubleRow against a constant 4.0 column (matching the 4x
    pre-scale of V), interleaved into a second PSUM bank; the final ACT copy
    applies the reciprocal as a per-partition scale.
  - The out stage is software-pipelined one block behind scores/exp so the
    ACT/DVE queues never stall the PE.
"""

from contextlib import ExitStack

import ml_dtypes
import numpy as np

import concourse.bass as bass
import concourse.bacc as bacc
import concourse.mybir as mybir
import concourse.tile as tile
from concourse.bass_utils import run_bass_kernel_spmd

P = 128
D = 512          # d_query == d_audio == d_out
CD = D // P      # 4 chunks of the feature dim
HW = 4096        # queries per batch
S = 1024         # keys per batch
SC = S // P      # 8 s-chunks
HWB = 512        # hw rows processed per block
NBLK = HW // HWB
B_FULL = 16
N_CORES = 8
BL = B_FULL // N_CORES  # 2 batches per core
SCALE = 1.0 / float(np.sqrt(D))
LN8 = float(np.log(8.0))

f32 = mybir.dt.float32
bf16 = mybir.dt.bfloat16
e4 = mybir.dt.float8e4
AFT = mybir.ActivationFunctionType
ALU = mybir.AluOpType
DR = mybir.MatmulPerfMode.DoubleRow

BF16NP = ml_dtypes.bfloat16


def build_nc():
    nc = bacc.Bacc("TRN2", target_bir_lowering=False, debug=False)

    x = nc.dram_tensor("x", [BL, HW, D], bf16, kind="ExternalInput").ap()
    audio = nc.dram_tensor("audio_embed", [BL, S, D], bf16, kind="ExternalInput").ap()
    wq = nc.dram_tensor("Wq", [D, D], bf16, kind="ExternalInput").ap()
    bq = nc.dram_tensor("bq", [D], f32, kind="ExternalInput").ap()
    wk = nc.dram_tensor("Wk", [D, D], bf16, kind="ExternalInput").ap()
    bk = nc.dram_tensor("bk", [D], f32, kind="ExternalInput").ap()
    wv = nc.dram_tensor("Wv", [D, D], bf16, kind="ExternalInput").ap()
    bv = nc.dram_tensor("bv", [D], bf16, kind="ExternalInput").ap()
    out = nc.dram_tensor("out", [BL, HW, D], f32, kind="ExternalOutput").ap()

    with tile.TileContext(nc) as tc:
        with ExitStack() as ctx:
            _body(ctx, tc, x, audio, wq, bq, wk, bk, wv, bv, out)

    nc.compile()
    return nc


def _body(ctx, tc, x, audio, wq, bq, wk, bk, wv, bv, out):
    nc = tc.nc

    const_pool = ctx.enter_context(tc.tile_pool(name="const", bufs=1))
    batch_pool = ctx.enter_context(tc.tile_pool(name="batch", bufs=2))
    work_pool = ctx.enter_context(tc.tile_pool(name="work", bufs=2))
    small_pool = ctx.enter_context(tc.tile_pool(name="small", bufs=4))
    psum_mm = ctx.enter_context(tc.tile_pool(name="pmm", bufs=3, space="PSUM"))
    psum_sc = ctx.enter_context(tc.tile_pool(name="psc", bufs=2, space="PSUM"))
    psum_den = ctx.enter_context(tc.tile_pool(name="pden", bufs=2, space="PSUM"))
    psum_dl = ctx.enter_context(tc.tile_pool(name="pdl", bufs=1, space="PSUM"))

    # Weight/bias loads are emitted lazily (after the first audio-half DMA)
    # so the first transposable input data leads the serial DMA queue.
    consts = {}

    def _load_consts():
        bk_sb = const_pool.tile([P, CD], f32)
        nc.sync.dma_start(bk_sb, bk.rearrange("(c p) -> p c", p=P))
        bq_f = const_pool.tile([P, CD], f32)
        nc.sync.dma_start(bq_f, bq.rearrange("(c p) -> p c", p=P))
        bq8 = const_pool.tile([P, CD], e4)
        nc.vector.tensor_copy(bq8, bq_f)
        bv_row = const_pool.tile([1, D], bf16)
        nc.sync.dma_start(bv_row, bv[None, :])
        ones_row = const_pool.tile([1, P], bf16)
        nc.gpsimd.memset(ones_row, 1.0)
        fours = const_pool.tile([P, 2, 1], e4)
        nc.gpsimd.memset(fours, 4.0)
        wk_sb = const_pool.tile([P, CD, D], bf16)
        nc.sync.dma_start(wk_sb, wk.rearrange("(c p) n -> p c n", p=P))
        wv_sb = const_pool.tile([P, CD, D], bf16)
        nc.sync.dma_start(wv_sb, wv.rearrange("(c p) n -> p c n", p=P))
        consts.update(wk_sb=wk_sb, wv_sb=wv_sb, bk_sb=bk_sb, bq8=bq8,
                      bv_row=bv_row, ones_row=ones_row, fours=fours)

    def _load_wq():
        wq_sb = const_pool.tile([P, CD, D], bf16)
        nc.sync.dma_start(wq_sb, wq.rearrange("(c p) n -> p c n", p=P))
        consts.update(wq_sb=wq_sb)

    def _emit_out(pend):
        exlh, vhl, b, blk = pend
        out_sb = work_pool.tile([P, CD, D], f32, tag="o")
        for h in range(CD):
            hs = slice(h * P, (h + 1) * P)
            num_ps = psum_mm.tile([P, D], f32, tag="mm")
            den_ps = psum_den.tile([P, 1], f32, tag="den")
            # hi*hi over k-tile pairs
            for t in range(SC // 2):
                nc.tensor.matmul(
                    num_ps,
                    exlh[:, 1, 2 * t : 2 * t + 2, hs],
                    vhl[:, 0, 2 * t : 2 * t + 2, :],
                    start=(t == 0),
                    stop=False,
                    perf_mode=DR,
                )
            # cross terms: slot0 = exl*vh, slot1 = exh*vl; den interleaved
            for t in range(SC):
                nc.tensor.matmul(
                    num_ps,
                    exlh[:, :, t, hs],
                    vhl[:, :, t, :],
                    start=False,
                    stop=(t == SC - 1),
                    perf_mode=DR,
                )
                nc.tensor.matmul(
                    den_ps,
                    exlh[:, :, t, hs],
                    consts["fours"],
                    start=(t == 0),
                    stop=(t == SC - 1),
                    perf_mode=DR,
                )
            rec = small_pool.tile([P, 1], f32, tag="rec")
            nc.vector.reciprocal(rec, den_ps)
            nc.scalar.activation(out_sb[:, h, :], num_ps, AFT.Copy, bias=0.0, scale=rec)
        nc.gpsimd.dma_start(
            out[b].rearrange("(t h p) n -> t p h n", p=P, h=CD)[blk], out_sb
        )

    x_pre = {}
    for b in range(BL):
        # --- per-batch: audio load+transpose, K-hi, V hi/lo, delta --------
        aT = batch_pool.tile([P, CD, S], bf16, tag="aT")
        kh = batch_pool.tile([P, CD, S], e4, tag="kh")
        vhl = batch_pool.tile([P, 2, SC, D], e4, tag="v")  # [:,0]=hi [:,1]=lo
        dT_ps = psum_dl.tile([P, SC], f32, tag="dl")
        dsb = batch_pool.tile([P, SC], f32, tag="dsb")
        for half in range(2):
            a_sb = work_pool.tile([P, CD, D], bf16, tag="a")
            nc.sync.dma_start(
                a_sb, audio[b].rearrange("(t c p) n -> t p c n", p=P, c=CD)[half]
            )
            if b == 0 and half == 0:
                _load_consts()
                x_pre0 = work_pool.tile([P, CD, D], bf16, tag="x", name="x_pre0")
                nc.sync.dma_start(
                    x_pre0, x[0].rearrange("(t c p) n -> t p c n", p=P, c=CD)[0]
                )
                x_pre[(0, 0)] = x_pre0
                _load_wq()
            for c in range(CD):
                sc_idx = half * CD + c
                nc.sync.dma_start_transpose(
                    aT[:, :, sc_idx * P : (sc_idx + 1) * P], a_sb[:, c, :]
                )
            hsl = slice(half * 512, (half + 1) * 512)
            for m in range(CD):
                mm_ps = psum_mm.tile([P, 512], f32, tag="mm")
                for c in range(CD):
                    nc.tensor.matmul(
                        mm_ps,
                        consts["wk_sb"][:, c, m * P : (m + 1) * P],
                        aT[:, c, hsl],
                        start=(c == 0),
                        stop=(c == CD - 1),
                    )
                nc.scalar.activation(
                    kh[:, m, hsl], mm_ps, AFT.Identity,
                    bias=consts["bk_sb"][:, m, None], scale=1.0,
                )
            for g in range(half * 4, half * 4 + 4):
                mm_ps = psum_mm.tile([P, D], f32, tag="mm")
                for c in range(CD):
                    nc.tensor.matmul(
                        mm_ps,
                        aT[:, c, g * P : (g + 1) * P],
                        consts["wv_sb"][:, c, :],
                        start=(c == 0),
                        stop=False,
                    )
                nc.tensor.matmul(
                    mm_ps, consts["ones_row"], consts["bv_row"],
                    start=False, stop=True,
                )
                nc.scalar.activation(vhl[:, 0, g, :], mm_ps, AFT.Copy)
                nc.vector.tensor_tensor(
                    vhl[:, 1, g, :], mm_ps, vhl[:, 0, g, :], ALU.subtract
                )
            # delta[s] = bq . K'[s] for this half's s-chunks (tiny DoubleRow)
            for g in range(half * 4, half * 4 + 4):
                for t in range(2):
                    nc.tensor.matmul(
                        dT_ps[:, g : g + 1],
                        kh[:, 2 * t : 2 * t + 2, g * P : (g + 1) * P],
                        consts["bq8"][:, 2 * t : 2 * t + 2, None],
                        start=(t == 0),
                        stop=(t == 1),
                        perf_mode=DR,
                    )
        # dsb = (SCALE/4) * dT + ln(8): exp-stage per-partition bias
        nc.vector.tensor_scalar(dsb, dT_ps, SCALE / 4.0, LN8, ALU.mult, ALU.add)

        # --- hw blocks; out-stage pipelined one block behind --------------
        pend = None
        for blk in range(NBLK):
            x_sb = x_pre.pop((b, blk), None)
            if x_sb is None:
                x_sb = work_pool.tile([P, CD, D], bf16, tag="x")
                nc.sync.dma_start(
                    x_sb, x[b].rearrange("(t c p) n -> t p c n", p=P, c=CD)[blk]
                )
            xT = work_pool.tile([P, CD, HWB], bf16, tag="xT")
            for c in range(CD):
                nc.sync.dma_start_transpose(
                    xT[:, :, c * P : (c + 1) * P], x_sb[:, c, :]
                )

            qh = work_pool.tile([P, CD, HWB], e4, tag="qh")
            for m in range(CD):
                mm_ps = psum_mm.tile([P, HWB], f32, tag="mm")
                for c in range(CD):
                    nc.tensor.matmul(
                        mm_ps,
                        consts["wq_sb"][:, c, m * P : (m + 1) * P],
                        xT[:, c, :],
                        start=(c == 0),
                        stop=(c == CD - 1),
                    )
                nc.scalar.activation(qh[:, m, :], mm_ps, AFT.Copy)

            exlh = work_pool.tile([P, 2, SC, HWB], e4, tag="ex")  # lo, hi
            for g in range(SC):
                sc_ps = psum_sc.tile([P, HWB], f32, tag="sc")
                for t in range(2):
                    nc.tensor.matmul(
                        sc_ps,
                        kh[:, 2 * t : 2 * t + 2, g * P : (g + 1) * P],
                        qh[:, 2 * t : 2 * t + 2, :],
                        start=(t == 0),
                        stop=(t == 1),
                        perf_mode=DR,
                    )
                ex_f = small_pool.tile([P, HWB], f32, tag="exf")
                nc.scalar.activation(
                    ex_f, sc_ps, AFT.Exp, bias=dsb[:, g, None], scale=SCALE / 16.0
                )
                nc.vector.tensor_copy(exlh[:, 1, g, :], ex_f)
                nc.vector.tensor_tensor(
                    exlh[:, 0, g, :], ex_f, exlh[:, 1, g, :], ALU.subtract
                )

            if pend is not None:
                _emit_out(pend)
            pend = (exlh, vhl, b, blk)
        _emit_out(pend)


_NC_CACHE = None


def _get_nc():
    global _NC_CACHE
    if _NC_CACHE is None:
        _NC_CACHE = build_nc()
    return _NC_CACHE


def make_in_maps(inputs):
    """Host-side prep: bf16 casts + 4x scaling of W/bk/bv, per-core slices."""
    x = np.asarray(inputs["x"], dtype=np.float32)
    audio = np.asarray(inputs["audio_embed"], dtype=np.float32)
    wq = (np.asarray(inputs["Wq"], dtype=np.float32) * 4.0).astype(BF16NP)
    bq = np.ascontiguousarray(np.asarray(inputs["bq"], dtype=np.float32))
    wk = (np.asarray(inputs["Wk"], dtype=np.float32) * 4.0).astype(BF16NP)
    bk = np.ascontiguousarray(np.asarray(inputs["bk"], dtype=np.float32) * 4.0)
    wv = (np.asarray(inputs["Wv"], dtype=np.float32) * 4.0).astype(BF16NP)
    bv = (np.asarray(inputs["bv"], dtype=np.float32) * 4.0).astype(BF16NP)
    xb = x.astype(BF16NP)
    ab = audio.astype(BF16NP)
    in_maps = []
    for i in range(N_CORES):
        in_maps.append(
            {
                "x": np.ascontiguousarray(xb[i * BL : (i + 1) * BL]),
                "audio_embed": np.ascontiguousarray(ab[i * BL : (i + 1) * BL]),
                "Wq": wq,
                "bq": bq,
                "Wk": wk,
                "bk": bk,
                "Wv": wv,
                "bv": bv,
            }
        )
    return in_maps


def kernel(**inputs):
    nc = _get_nc()
    in_maps = make_in_maps(inputs)
    res = run_bass_kernel_spmd(nc, in_maps, core_ids=list(range(N_CORES)))
    return np.concatenate([res.results[i]["out"] for i in range(N_CORES)], axis=0)
